# revision 54
# baseline (speedup 1.0000x reference)
# Multi-head attention (B=2, S=2048, D=1024, H=16, d=64) on 8 NeuronCores.
#
# Sharding: core c handles batch b = c//4 and head group g = c%4 (4 heads).
# Column-shard qw/kw/vw by head group, row-shard ow; partial outputs are
# summed on the host per batch.
#
# Per-core dataflow (everything in transposed [feature, seq] layout):
#   Q^T = wq_g.T @ x^T   (f32r matmul, wq pre-scaled by 1/sqrt(D) on host)
#   K^T = wk_g.T @ x^T   (f32r)
#   V   = x @ wv_g       (f32r, natural [seq, d] layout, + ones column, bf16)
#   RoPE on head-PAIR tiles [128, S] (cos/sin tables sign-folded on host)
#   pass 1: S[q,k] tiles in bf16, two heads concurrently on PE row groups
#     (lhsT base partitions 0/64) -> row max m[q] via DVE reduce
#   m column -> row via PE transpose with a negated identity, bounced
#     through DRAM into row 64 of the extended Q operand
#   pass 2: S^T[k,q] = [K^T;1].T @ [Q^T;-m]  (f32r, 65-dim contraction folds
#     the max subtraction into the matmul) -> exp on ACT -> P^T bf16 in SBUF
#   PV: [V|1].T @ P^T accumulated over k tiles (bf16, full PE rate);
#     row 64 gives the softmax denominator l[q] for free
#   merged^T = attn^T * (1/l)  (1/l on [1,512], replicated via f32r matmul)
#   out_partial = merged^T.T @ ow_g (f32r)

import ml_dtypes
import numpy as np

_STATE = {}

B, S, D, H, HD = 2, 2048, 1024, 16, 64
HPC = 4          # heads per core
GC = HPC * HD    # columns per core = 256
NKC = D // 128   # k chunks for d_model contraction = 8
NST = S // 128   # seq tiles = 16
NSC = S // 512   # seq chunks of 512 = 4


def _build():
    import concourse.tile as tile
    import concourse.mybir as mybir
    from concourse import bacc
    from concourse.masks import make_identity

    f32 = mybir.dt.float32
    f32r = mybir.dt.float32r
    bf16 = mybir.dt.bfloat16

    nc = bacc.Bacc(None, target_bir_lowering=False, debug=False)

    xt_d = nc.dram_tensor("xt", [D, S], f32r, kind="ExternalInput")
    wq_d = nc.dram_tensor("wq", [D, GC], f32r, kind="ExternalInput")
    wk_d = nc.dram_tensor("wk", [D, GC], f32r, kind="ExternalInput")
    wv_d = nc.dram_tensor("wv", [D, GC], f32r, kind="ExternalInput")
    wo_d = nc.dram_tensor("wo", [GC, D], bf16, kind="ExternalInput")
    ones_d = nc.dram_tensor("onesrow", [1, S], f32r, kind="ExternalInput")
    cos_d = nc.dram_tensor("cos64", [128, S], f32, kind="ExternalInput")
    sin_d = nc.dram_tensor("sin64", [128, S], f32, kind="ExternalInput")
    out_d = nc.dram_tensor("out", [S, D], f32, kind="ExternalOutput")

    with tile.TileContext(nc) as tc:
        with (
            tc.tile_pool(name="sb", bufs=1) as sb,
            tc.tile_pool(name="dram", bufs=2, space="DRAM") as dram,
            tc.tile_pool(name="ps", bufs=2, space="PSUM") as psp,
        ):
            # ---- setup tiles ----
            v_sb = sb.tile([128, NST, HPC, HD + 1], bf16, tag="v", name="v_sb")
            onescol = sb.tile([128, 1], bf16, tag="onescol", name="onescol")
            nc.vector.memset(onescol, 1.0)
            nc.vector.tensor_copy(
                out=v_sb[:, :, :, HD],
                in_=onescol.to_broadcast([128, NST, HPC]),
            )
            ident = sb.tile([128, 128], f32, tag="idn", name="ident")
            make_identity(nc, ident)

            wq_sb = sb.tile([128, NKC, GC], f32r, tag="wq", name="wq_sb")
            wk_sb = sb.tile([128, NKC, GC], f32r, tag="wk", name="wk_sb")
            wv_sb = sb.tile([128, NKC, GC], f32r, tag="wv", name="wv_sb")
            for w_sb, w_d in ((wq_sb, wq_d), (wk_sb, wk_d), (wv_sb, wv_d)):
                nc.sync.dma_start(
                    out=w_sb, in_=w_d[:].rearrange("(kc p) m -> p kc m", p=128)
                )
            # cos/sin (needed at rope) and wo (needed at the end) load later
            # so the first xt chunks aren't stuck behind them in the queue
            cos_t = sb.tile([128, S], f32, tag="cos", name="cos_t")
            sin_t = sb.tile([128, S], f32, tag="sin", name="sin_t")
            wo_sb = sb.tile([128, 2, D], bf16, tag="wo", name="wo_sb")

            def late_loads():
                nc.sync.dma_start(out=cos_t, in_=cos_d[:])
                nc.sync.dma_start(out=sin_t, in_=sin_d[:])
                nc.sync.dma_start(
                    out=wo_sb, in_=wo_d[:].rearrange("(c p) n -> p c n", p=128)
                )

            ones1 = sb.tile([1, HD], f32r, tag="ones1", name="ones1")
            nc.sync.dma_start(out=ones1, in_=ones_d[:, 0:HD])

            mrg = [None, None]
            for ch in range(2):
                mrg[ch] = sb.tile([128, S], bf16, tag="mrg", bufs=2,
                                  name=f"mrg{ch}")

            def project_chunks(grp):
                """Q^T/K^T for head pair `grp` (and V for all heads on grp 0).
                Returns (qpre, kpre, [emit_sc closures])."""
                qpre = sb.tile([128, S], f32, tag="pre", bufs=2, name=f"qp{grp}")
                kpre = sb.tile([128, S], f32, tag="pre", bufs=2, name=f"kp{grp}")

                def emit_sc(sc):
                    xt_sc = []
                    for kc in range(NKC):
                        xx = sb.tile([128, 512], f32r, tag="xt", bufs=12,
                                     name=f"xt{grp}_{sc}_{kc}")
                        nc.sync.dma_start(
                            out=xx,
                            in_=xt_d[
                                kc * 128 : (kc + 1) * 128,
                                sc * 512 : (sc + 1) * 512,
                            ],
                        )
                        xt_sc.append(xx)
                    for w_sb, pre in ((wq_sb, qpre), (wk_sb, kpre)):
                        ps = psp.tile([128, 512], f32, tag="s1", bufs=2,
                                      name="ps_qk")
                        for kc in range(NKC):
                            nc.tensor.matmul(
                                ps,
                                lhsT=w_sb[:, kc, grp * 128 : (grp + 1) * 128],
                                rhs=xt_sc[kc],
                                start=(kc == 0),
                                stop=(kc == NKC - 1),
                            )
                        nc.scalar.copy(
                            out=pre[:, sc * 512 : (sc + 1) * 512], in_=ps
                        )
                    if grp == 0:
                        for st4 in range(4):
                            st = sc * 4 + st4
                            ps = psp.tile([128, 512], f32, tag="s1", bufs=2,
                                          name="ps_v")
                            psv = ps[:, :GC]
                            for kc in range(NKC):
                                nc.tensor.matmul(
                                    psv,
                                    lhsT=xt_sc[kc][:, st4 * 128 : (st4 + 1) * 128],
                                    rhs=wv_sb[:, kc, :],
                                    start=(kc == 0),
                                    stop=(kc == NKC - 1),
                                )
                            nc.scalar.copy(
                                out=v_sb[:, st, :, 0:HD],
                                in_=psv.rearrange("p (h d) -> p h d", h=HPC),
                            )

                return qpre, kpre, [lambda sc=sc: emit_sc(sc) for sc in range(NSC)]

            def rope_pair(grp):
                """RoPE both heads of the pair at once on [128, S] tiles.

                Returns (tiles, emit_sc, ones_k) where emit_sc(sc, ...) is an
                sc-chunk closure emitted right after the projection chunk
                that feeds it, and ones_k() fills ext2k row 64 with ones —
                emitted only AFTER pass 1 (which reads that row as the odd
                head's first feature).
                tiles = (ext2q, ext2k, qx1, kx1):
                  ext2q/ext2k: roped f32r, head even in rows 0:64 (row 64
                    later becomes the even head's -m / ones row in place),
                    head odd in rows 64:128 — read directly by pass 1
                    (f32r matmuls run at full rate for N=512)
                  qx1/kx1: odd head's [HD+1, S] operands (rows via DMA)
                """
                ext2q = sb.tile([128, S], f32r, tag="ext", bufs=4,
                                name=f"exq{grp}")
                ext2k = sb.tile([128, S], f32r, tag="ext", bufs=4,
                                name=f"exk{grp}")
                qx1 = sb.tile([HD + 1, S], f32r, tag="qx", bufs=4,
                              name=f"qx1_{grp}")
                kx1 = sb.tile([HD + 1, S], f32r, tag="qx", bufs=4,
                              name=f"kx1_{grp}")
                sw0 = sb.tile([128, S], f32, tag="sw", bufs=1,
                              name=f"sw{grp}")
                sw = [sw0, sw0]

                def emit_sc(sc, qpre, kpre):
                    cl = slice(sc * 512, (sc + 1) * 512)
                    for i, (pre, ext, x1) in enumerate((
                        (qpre, ext2q, qx1),
                        (kpre, ext2k, kx1),
                    )):
                        for half in range(2):
                            o = 64 * half
                            nc.sync.dma_start(
                                out=sw[i][o : o + 32, cl],
                                in_=pre[o + 32 : o + 64, cl],
                            )
                            nc.sync.dma_start(
                                out=sw[i][o + 32 : o + 64, cl],
                                in_=pre[o : o + 32, cl],
                            )
                        nc.vector.tensor_mul(out=ext[:, cl], in0=pre[:, cl],
                                             in1=cos_t[:, cl])
                        nc.vector.tensor_mul(out=sw[i][:, cl], in0=sw[i][:, cl],
                                             in1=sin_t[:, cl])
                        nc.vector.tensor_add(out=ext[:, cl], in0=ext[:, cl],
                                             in1=sw[i][:, cl])
                        # odd head's pass-2 operand (cross-partition -> DMA)
                        nc.sync.dma_start(out=x1[0:HD, cl], in_=ext[HD:128, cl])
                    if sc == NSC - 1:
                        nc.sync.dma_start(out=kx1[HD : HD + 1, :],
                                          in_=ones_d[:])

                def ones_k():
                    nc.sync.dma_start(out=ext2k[HD : HD + 1, :],
                                      in_=ones_d[:])

                tiles = (ext2q, ext2k, qx1, kx1)
                return tiles, emit_sc, ones_k

            def pass1_chunks(grp, ext2q, ext2k, qx_by_half):
                """bf16 scores for both heads of the pair, interleaved so the
                PE runs them concurrently on row groups 0-1 / 2-3 (lhsT base
                partitions 0 / 64). Row max -> -m into row 64 of each head's
                extended-Q operand. Returns [4 qt-quarter closures, finalize]."""
                mpart = [
                    sb.tile([128, NST, 2], f32, tag="mpart", bufs=4,
                            name=f"mp{grp}_{half}")
                    for half in range(2)
                ]

                def emit_sub(q4, cp):
                    for qt in range(q4 * 4, q4 * 4 + 4):
                        ps2 = []
                        for half in range(2):
                            off = 64 * half
                            ps = psp.tile([128, 1024], f32, tag="s1",
                                          bufs=2, name=f"ps_s1_{half}")
                            for c2 in range(2):
                                c = cp * 2 + c2
                                nc.tensor.matmul(
                                    ps[:, c2 * 512 : (c2 + 1) * 512],
                                    lhsT=ext2q[off : off + HD,
                                               qt * 128 : (qt + 1) * 128],
                                    rhs=ext2k[off : off + HD,
                                              c * 512 : (c + 1) * 512],
                                    start=True,
                                    stop=True,
                                )
                            ps2.append(ps)
                        for half in range(2):
                            nc.vector.reduce_max(
                                out=mpart[half][:, qt, cp : cp + 1],
                                in_=ps2[half],
                                axis=mybir.AxisListType.X,
                            )

                def finalize(hq):
                    """-m for q-tiles [hq*8, hq*8+8) of both heads — lets
                    pass 2's qh=hq chunks start after half the reduces."""
                    qsl = slice(hq * 8, hq * 8 + 8)
                    for half in range(2):
                        mcol = sb.tile([128, 8], f32, tag="mcol", bufs=4,
                                       name=f"mc{grp}_{half}_{hq}")
                        nc.vector.reduce_max(out=mcol,
                                             in_=mpart[half][:, qsl, :],
                                             axis=mybir.AxisListType.X)
                        pst = psp.tile([8, 128], f32, tag="s1", bufs=2,
                                       name=f"tp{grp}_{half}_{hq}")
                        nc.tensor.transpose(pst, mcol, ident)
                        msb = sb.tile([8, 128], f32r, tag="msb", bufs=4,
                                      name=f"ms{grp}_{half}_{hq}")
                        nc.vector.tensor_scalar_mul(msb, pst, -1.0)
                        mrow_d = dram.tile([1, 1024], f32r, tag="mrow",
                                           name=f"mrow{grp}_{half}_{hq}")
                        nc.sync.dma_start(
                            out=mrow_d[:].rearrange("o (a b) -> (o a) b",
                                                    a=8),
                            in_=msb,
                        )
                        nc.sync.dma_start(
                            out=qx_by_half[half][HD : HD + 1,
                                                 hq * 1024 : (hq + 1) * 1024],
                            in_=mrow_d[:],
                        )

                blocks = {
                    (q4, cp): (lambda q4=q4, cp=cp: emit_sub(q4, cp))
                    for q4 in range(4)
                    for cp in range(2)
                }
                blocks["finA"] = lambda: finalize(0)
                blocks["finB"] = lambda: finalize(1)
                return blocks

            def pass2_chunks(h, qx, kx, recb_on_act=False):
                """f32r S^T -> exp -> bf16 P^T -> PV (bf16) -> merged^T.
                Returns [2 qh-half closures]."""
                ch, offr = h // 2, 64 * (h % 2)

                state = {}

                def emit_sc(qh, kb):
                    # scores + exp for 4 k-tiles; their PV matmuls are a
                    # separate closure so another head's scores can slot in
                    # between (keeps ACT fed while PE does PV work)
                    if kb == 0:
                        state[qh] = [
                            psp.tile([128, 512], f32, tag="pv", bufs=2,
                                     name=f"pv{h}_{qh}_{c}")
                            for c in range(2)
                        ]
                    pts = []
                    for kt4 in range(4):
                        kt = kb * 4 + kt4
                        pt = sb.tile([128, 1024], bf16, tag="pt", bufs=10,
                                     name=f"pt{h}_{qh}_{kt}")
                        for c in range(2):
                            q0 = qh * 1024 + c * 512
                            ps = psp.tile([128, 512], f32, tag="s2",
                                          bufs=2, name="ps_s2")
                            nc.tensor.matmul(
                                ps,
                                lhsT=kx[:, kt * 128 : (kt + 1) * 128],
                                rhs=qx[:, q0 : q0 + 512],
                                start=True,
                                stop=True,
                            )
                            nc.scalar.activation(
                                out=pt[:, c * 512 : (c + 1) * 512],
                                in_=ps,
                                func=mybir.ActivationFunctionType.Exp,
                            )
                        pts.append((kt, pt))
                    state[(qh, kb)] = pts

                def emit_pv(qh, kb):
                    pv = state[qh]
                    for kt, pt in state.pop((qh, kb)):
                        for c in range(2):
                            nc.tensor.matmul(
                                pv[c][: HD + 1, :],
                                lhsT=v_sb[:, kt, h, :],
                                rhs=pt[:, c * 512 : (c + 1) * 512],
                                start=(kt == 0),
                                stop=(kt == NST - 1),
                            )

                def emit_fin(qh, c):
                    pv = state[qh]
                    q0 = qh * 1024 + c * 512
                    recb = sb.tile([HD, 512], f32r, tag="recb", bufs=2,
                                   name=f"rb{h}_{qh}_{c}")
                    with nc.allow_low_precision(
                        reason="1/l in f32r (19-bit) is plenty"
                    ):
                        nc.vector.reciprocal(
                            out=recb[0:1, :],
                            in_=pv[c][HD : HD + 1, :],
                        )
                    # replicate 1/l across 64 partitions (f32r matmul)
                    lps = psp.tile([HD, 512], f32, tag="s2", bufs=2,
                                   name=f"lp{h}_{qh}_{c}")
                    nc.tensor.matmul(lps, lhsT=ones1,
                                     rhs=recb[0:1, :],
                                     start=True, stop=True)
                    if recb_on_act:
                        nc.scalar.copy(out=recb, in_=lps)
                    else:
                        nc.vector.tensor_copy(out=recb, in_=lps)
                    nc.vector.tensor_mul(
                        out=mrg[ch][offr : offr + HD, q0 : q0 + 512],
                        in0=pv[c][0:HD, :],
                        in1=recb,
                    )

                return [
                    {
                        "sc": [lambda qh=qh, kb=kb: emit_sc(qh, kb)
                               for kb in range(NST // 4)],
                        "pv": [lambda qh=qh, kb=kb: emit_pv(qh, kb)
                               for kb in range(NST // 4)],
                        "fin": [lambda qh=qh, c=c: emit_fin(qh, c)
                                for c in range(2)],
                    }
                    for qh in range(2)
                ]

            def outproj_qt(qt, obuf_on_act=False):
                """one q-tile of the output projection (bf16 matmul)."""
                if True:
                    for n in range(2):
                        ps = psp.tile([128, 512], f32, tag="s1", bufs=2,
                                      name="ps_o")
                        for ch in range(2):
                            nc.tensor.matmul(
                                ps,
                                lhsT=mrg[ch][:, qt * 128 : (qt + 1) * 128],
                                rhs=wo_sb[:, ch, n * 512 : (n + 1) * 512],
                                start=(ch == 0),
                                stop=(ch == 1),
                            )
                        obuf = sb.tile([128, 512], f32, tag="obuf", bufs=2,
                                       name=f"ob{qt}_{n}")
                        if obuf_on_act:
                            nc.scalar.copy(out=obuf, in_=ps)
                        else:
                            nc.vector.tensor_copy(out=obuf, in_=ps)
                        nc.sync.dma_start(
                            out=out_d[qt * 128 : (qt + 1) * 128,
                                      n * 512 : (n + 1) * 512],
                            in_=obuf,
                        )

            # ---- pipeline ----
            # Emission order approximates the per-engine schedule with a
            # fine-grained round-robin: pass-1 sub-blocks (4 q-tiles, DVE
            # reduce work) alternate with projection/rope chunks and with
            # pass-2 kb batches (PE scores + ACT exp + PE PV), so no engine
            # waits long behind another's FIFO.
            qp0, kp0, proj0 = project_chunks(0)
            t0, rope0, ones_k0 = rope_pair(0)
            ext2q0, ext2k0, qx10, kx10 = t0
            proj0[0]()
            late_loads()
            rope0(0, qp0, kp0)
            proj0[1]()
            rope0(1, qp0, kp0)
            p1_0 = pass1_chunks(0, ext2q0, ext2k0, (ext2q0, qx10))
            qp1, kp1, proj1 = project_chunks(1)
            t1, rope1, ones_k1 = rope_pair(1)
            ext2q1, ext2k1, qx11, kx11 = t1
            # (q4,cp)=(*,0) needs only ext cols 0:1024 (sc 0-1)
            p1_0[(0, 0)]()
            p1_0[(1, 0)]()
            proj0[2]()
            rope0(2, qp0, kp0)
            p1_0[(2, 0)]()
            proj0[3]()
            rope0(3, qp0, kp0)
            p1_0[(3, 0)]()
            p1_0[(0, 1)]()
            proj1[0]()
            rope1(0, qp1, kp1)
            p1_0[(1, 1)]()
            p1_0["finA"]()
            proj1[1]()
            rope1(1, qp1, kp1)
            p1_0[(2, 1)]()
            proj1[2]()
            rope1(2, qp1, kp1)
            p1_0[(3, 1)]()
            p1_0["finB"]()
            ones_k0()  # ext2k0 row 64 -> ones (after pass 1 reads it)
            proj1[3]()
            rope1(3, qp1, kp1)

            p1_1 = pass1_chunks(1, ext2q1, ext2k1, (ext2q1, qx11))
            p2_0a = pass2_chunks(1, qx10, kx10, recb_on_act=True)
            p2_0b = pass2_chunks(0, ext2q0[0 : HD + 1, :],
                                 ext2k0[0 : HD + 1, :], recb_on_act=True)

            def unit(p, qh, fins=True):
                """software-pipelined (head, qh) group: scores run one kb
                batch ahead of their PV matmuls so PE never waits on exp"""
                sc, pv = p[qh]["sc"], p[qh]["pv"]
                seq = [sc[0], sc[1], pv[0], sc[2], pv[1], sc[3], pv[2], pv[3]]
                return seq + (p[qh]["fin"] if fins else [])

            # head 1's qh0 unit is ready as soon as finA lands — emit it
            # before grp1's pass 1 so its exps fill the ACT-idle lead-in
            for emit in unit(p2_0a, 0):
                emit()
            flatB = (unit(p2_0b, 0) + unit(p2_0a, 1) + unit(p2_0b, 1))
            subs1 = [(0, 0), (1, 0), (0, 1), (1, 1), "finA",
                     (2, 0), (3, 0), (2, 1), (3, 1), "finB"]
            fi = 0
            for key in subs1:
                p1_1[key]()
                if not isinstance(key, str):
                    n = 4 if fi < 8 else 3
                    for emit in flatB[fi : fi + n]:
                        emit()
                    fi += n
            for emit in flatB[fi:]:
                emit()
            ones_k1()  # ext2k1 row 64 -> ones (after pass 1 reads it)

            p2_1a = pass2_chunks(3, qx11, kx11)
            p2_1b = pass2_chunks(2, ext2q1[0 : HD + 1, :],
                                 ext2k1[0 : HD + 1, :])
            for emit in unit(p2_1a, 0) + unit(p2_1b, 0):
                emit()
            flatC = unit(p2_1a, 1) + unit(p2_1b, 1)
            for i in range(10):
                flatC[2 * i]()
                flatC[2 * i + 1]()
                if i < 8:
                    outproj_qt(i)
            for qt in range(8, NST):
                outproj_qt(qt, obuf_on_act=True)

    nc.compile()
    return nc


def _tables():
    j = np.arange(0, HD, 2, dtype=np.float32)
    inv_freq = (
        np.float32(1.0) / (np.float32(10000.0) ** (j / np.float32(HD)))
    ).astype(np.float32)
    freqs = np.arange(S, dtype=np.float32)[:, None] * inv_freq[None, :]  # [S, 32]
    cos = np.cos(freqs).astype(np.float32).T  # [32, S]
    sin = np.sin(freqs).astype(np.float32).T
    cos128 = np.concatenate([cos, cos, cos, cos], axis=0)  # [128, S]
    sin128 = np.concatenate([-sin, sin, -sin, sin], axis=0)
    return np.ascontiguousarray(cos128), np.ascontiguousarray(sin128)


def kernel(x, qw, kw, vw, ow):
    from concourse.bass_utils import run_bass_kernel_spmd

    if "nc" not in _STATE:
        _STATE["nc"] = _build()
    nc = _STATE["nc"]

    x = np.asarray(x, dtype=np.float32)
    qw = np.asarray(qw, dtype=np.float32)
    kw = np.asarray(kw, dtype=np.float32)
    vw = np.asarray(vw, dtype=np.float32)
    ow = np.asarray(ow, dtype=np.float32)

    cos64, sin64 = _tables()
    scale = np.float32(1.0 / 32.0)  # 1/sqrt(D), exact power of two

    in_maps = []
    for c in range(8):
        b, g = c // 4, c % 4
        sl = slice(g * GC, (g + 1) * GC)
        in_maps.append(
            {
                "xt": np.ascontiguousarray(x[b].T),
                "wq": np.ascontiguousarray(qw[:, sl]) * scale,
                "wk": np.ascontiguousarray(kw[:, sl]),
                "wv": np.ascontiguousarray(vw[:, sl]),
                "wo": np.ascontiguousarray(ow[sl, :]).astype(ml_dtypes.bfloat16),
                "onesrow": np.ones((1, S), dtype=np.float32),
                "cos64": cos64,
                "sin64": sin64,
            }
        )

    res = run_bass_kernel_spmd(nc, in_maps, core_ids=list(range(8)))
    _STATE["last_res"] = res
    outs = [r["out"] for r in res.results]
    full = np.empty((B, S, D), dtype=np.float32)
    for b in range(B):
        full[b] = sum(o.astype(np.float64) for o in outs[4 * b : 4 * b + 4]).astype(
            np.float32
        )
    return full


# revision 55
# speedup vs baseline: 1.0028x; 1.0028x over previous
# Multi-head attention (B=2, S=2048, D=1024, H=16, d=64) on 8 NeuronCores.
#
# Sharding: core c handles batch b = c//4 and head group g = c%4 (4 heads).
# Column-shard qw/kw/vw by head group, row-shard ow; partial outputs are
# summed on the host per batch.
#
# Per-core dataflow (everything in transposed [feature, seq] layout):
#   Q^T = wq_g.T @ x^T   (f32r matmul, wq pre-scaled by 1/sqrt(D) on host)
#   K^T = wk_g.T @ x^T   (f32r)
#   V   = x @ wv_g       (f32r, natural [seq, d] layout, + ones column, bf16)
#   RoPE on head-PAIR tiles [128, S] (cos/sin tables sign-folded on host)
#   pass 1: S[q,k] tiles in bf16, two heads concurrently on PE row groups
#     (lhsT base partitions 0/64) -> row max m[q] via DVE reduce
#   m column -> row via PE transpose with a negated identity, bounced
#     through DRAM into row 64 of the extended Q operand
#   pass 2: S^T[k,q] = [K^T;1].T @ [Q^T;-m]  (f32r, 65-dim contraction folds
#     the max subtraction into the matmul) -> exp on ACT -> P^T bf16 in SBUF
#   PV: [V|1].T @ P^T accumulated over k tiles (bf16, full PE rate);
#     row 64 gives the softmax denominator l[q] for free
#   merged^T = attn^T * (1/l)  (1/l on [1,512], replicated via f32r matmul)
#   out_partial = merged^T.T @ ow_g (f32r)

import ml_dtypes
import numpy as np

_STATE = {}

B, S, D, H, HD = 2, 2048, 1024, 16, 64
HPC = 4          # heads per core
GC = HPC * HD    # columns per core = 256
NKC = D // 128   # k chunks for d_model contraction = 8
NST = S // 128   # seq tiles = 16
NSC = S // 512   # seq chunks of 512 = 4


def _build():
    import concourse.tile as tile
    import concourse.mybir as mybir
    from concourse import bacc
    from concourse.masks import make_identity

    f32 = mybir.dt.float32
    f32r = mybir.dt.float32r
    bf16 = mybir.dt.bfloat16

    nc = bacc.Bacc(None, target_bir_lowering=False, debug=False)

    xt_d = nc.dram_tensor("xt", [D, S], f32r, kind="ExternalInput")
    wq_d = nc.dram_tensor("wq", [D, GC], f32r, kind="ExternalInput")
    wk_d = nc.dram_tensor("wk", [D, GC], f32r, kind="ExternalInput")
    wv_d = nc.dram_tensor("wv", [D, GC], f32r, kind="ExternalInput")
    wo_d = nc.dram_tensor("wo", [GC, D], bf16, kind="ExternalInput")
    ones_d = nc.dram_tensor("onesrow", [1, S], f32r, kind="ExternalInput")
    cos_d = nc.dram_tensor("cos64", [128, S], f32, kind="ExternalInput")
    sin_d = nc.dram_tensor("sin64", [128, S], f32, kind="ExternalInput")
    out_d = nc.dram_tensor("out", [S, D], f32, kind="ExternalOutput")

    with tile.TileContext(nc) as tc:
        with (
            tc.tile_pool(name="sb", bufs=1) as sb,
            tc.tile_pool(name="dram", bufs=2, space="DRAM") as dram,
            tc.tile_pool(name="ps", bufs=2, space="PSUM") as psp,
        ):
            # ---- setup tiles ----
            v_sb = sb.tile([128, NST, HPC, HD + 1], bf16, tag="v", name="v_sb")
            onescol = sb.tile([128, 1], bf16, tag="onescol", name="onescol")
            nc.vector.memset(onescol, 1.0)
            nc.vector.tensor_copy(
                out=v_sb[:, :, :, HD],
                in_=onescol.to_broadcast([128, NST, HPC]),
            )
            ident = sb.tile([128, 128], f32, tag="idn", name="ident")
            make_identity(nc, ident)

            wq_sb = sb.tile([128, NKC, GC], f32r, tag="wq", name="wq_sb")
            wk_sb = sb.tile([128, NKC, GC], f32r, tag="wk", name="wk_sb")
            wv_sb = sb.tile([128, NKC, GC], f32r, tag="wv", name="wv_sb")
            for w_sb, w_d in ((wq_sb, wq_d), (wk_sb, wk_d), (wv_sb, wv_d)):
                nc.sync.dma_start(
                    out=w_sb, in_=w_d[:].rearrange("(kc p) m -> p kc m", p=128)
                )
            # cos/sin (needed at rope) and wo (needed at the end) load later
            # so the first xt chunks aren't stuck behind them in the queue
            cos_t = sb.tile([128, S], f32, tag="cos", name="cos_t")
            sin_t = sb.tile([128, S], f32, tag="sin", name="sin_t")
            wo_sb = sb.tile([128, 2, D], bf16, tag="wo", name="wo_sb")

            def late_loads():
                nc.sync.dma_start(out=cos_t, in_=cos_d[:])
                nc.sync.dma_start(out=sin_t, in_=sin_d[:])
                nc.sync.dma_start(
                    out=wo_sb, in_=wo_d[:].rearrange("(c p) n -> p c n", p=128)
                )

            ones1 = sb.tile([1, HD], f32r, tag="ones1", name="ones1")
            nc.sync.dma_start(out=ones1, in_=ones_d[:, 0:HD])

            mrg = [None, None]
            for ch in range(2):
                mrg[ch] = sb.tile([128, S], bf16, tag="mrg", bufs=2,
                                  name=f"mrg{ch}")

            def project_chunks(grp):
                """Q^T/K^T for head pair `grp` (and V for all heads on grp 0).
                Returns (qpre, kpre, [emit_sc closures])."""
                qpre = sb.tile([128, S], f32, tag="pre", bufs=2, name=f"qp{grp}")
                kpre = sb.tile([128, S], f32, tag="pre", bufs=2, name=f"kp{grp}")

                def emit_sc(sc):
                    xt_sc = []
                    for kc in range(NKC):
                        xx = sb.tile([128, 512], f32r, tag="xt", bufs=12,
                                     name=f"xt{grp}_{sc}_{kc}")
                        nc.sync.dma_start(
                            out=xx,
                            in_=xt_d[
                                kc * 128 : (kc + 1) * 128,
                                sc * 512 : (sc + 1) * 512,
                            ],
                        )
                        xt_sc.append(xx)
                    for w_sb, pre in ((wq_sb, qpre), (wk_sb, kpre)):
                        ps = psp.tile([128, 512], f32, tag="s1", bufs=2,
                                      name="ps_qk")
                        for kc in range(NKC):
                            nc.tensor.matmul(
                                ps,
                                lhsT=w_sb[:, kc, grp * 128 : (grp + 1) * 128],
                                rhs=xt_sc[kc],
                                start=(kc == 0),
                                stop=(kc == NKC - 1),
                            )
                        nc.scalar.copy(
                            out=pre[:, sc * 512 : (sc + 1) * 512], in_=ps
                        )
                    if grp == 0:
                        for st4 in range(4):
                            st = sc * 4 + st4
                            ps = psp.tile([128, 512], f32, tag="s1", bufs=2,
                                          name="ps_v")
                            psv = ps[:, :GC]
                            for kc in range(NKC):
                                nc.tensor.matmul(
                                    psv,
                                    lhsT=xt_sc[kc][:, st4 * 128 : (st4 + 1) * 128],
                                    rhs=wv_sb[:, kc, :],
                                    start=(kc == 0),
                                    stop=(kc == NKC - 1),
                                )
                            nc.scalar.copy(
                                out=v_sb[:, st, :, 0:HD],
                                in_=psv.rearrange("p (h d) -> p h d", h=HPC),
                            )

                return qpre, kpre, [lambda sc=sc: emit_sc(sc) for sc in range(NSC)]

            def rope_pair(grp):
                """RoPE both heads of the pair at once on [128, S] tiles.

                Returns (tiles, emit_sc, ones_k) where emit_sc(sc, ...) is an
                sc-chunk closure emitted right after the projection chunk
                that feeds it, and ones_k() fills ext2k row 64 with ones —
                emitted only AFTER pass 1 (which reads that row as the odd
                head's first feature).
                tiles = (ext2q, ext2k, qx1, kx1):
                  ext2q/ext2k: roped f32r, head even in rows 0:64 (row 64
                    later becomes the even head's -m / ones row in place),
                    head odd in rows 64:128 — read directly by pass 1
                    (f32r matmuls run at full rate for N=512)
                  qx1/kx1: odd head's [HD+1, S] operands (rows via DMA)
                """
                ext2q = sb.tile([128, S], f32r, tag="ext", bufs=4,
                                name=f"exq{grp}")
                ext2k = sb.tile([128, S], f32r, tag="ext", bufs=4,
                                name=f"exk{grp}")
                qx1 = sb.tile([HD + 1, S], f32r, tag="qx", bufs=4,
                              name=f"qx1_{grp}")
                kx1 = sb.tile([HD + 1, S], f32r, tag="qx", bufs=4,
                              name=f"kx1_{grp}")
                sw0 = sb.tile([128, S], f32, tag="sw", bufs=1,
                              name=f"sw{grp}")
                sw = [sw0, sw0]

                def emit_sc(sc, qpre, kpre):
                    cl = slice(sc * 512, (sc + 1) * 512)
                    for i, (pre, ext, x1) in enumerate((
                        (qpre, ext2q, qx1),
                        (kpre, ext2k, kx1),
                    )):
                        for half in range(2):
                            o = 64 * half
                            nc.sync.dma_start(
                                out=sw[i][o : o + 32, cl],
                                in_=pre[o + 32 : o + 64, cl],
                            )
                            nc.sync.dma_start(
                                out=sw[i][o + 32 : o + 64, cl],
                                in_=pre[o : o + 32, cl],
                            )
                        nc.vector.tensor_mul(out=ext[:, cl], in0=pre[:, cl],
                                             in1=cos_t[:, cl])
                        nc.vector.tensor_mul(out=sw[i][:, cl], in0=sw[i][:, cl],
                                             in1=sin_t[:, cl])
                        nc.vector.tensor_add(out=ext[:, cl], in0=ext[:, cl],
                                             in1=sw[i][:, cl])
                        # odd head's pass-2 operand (cross-partition -> DMA)
                        nc.sync.dma_start(out=x1[0:HD, cl], in_=ext[HD:128, cl])
                    if sc == NSC - 1:
                        nc.sync.dma_start(out=kx1[HD : HD + 1, :],
                                          in_=ones_d[:])

                def ones_k():
                    nc.sync.dma_start(out=ext2k[HD : HD + 1, :],
                                      in_=ones_d[:])

                tiles = (ext2q, ext2k, qx1, kx1)
                return tiles, emit_sc, ones_k

            def pass1_chunks(grp, ext2q, ext2k, qx_by_half):
                """bf16 scores for both heads of the pair, interleaved so the
                PE runs them concurrently on row groups 0-1 / 2-3 (lhsT base
                partitions 0 / 64). Row max -> -m into row 64 of each head's
                extended-Q operand. Returns [4 qt-quarter closures, finalize]."""
                mpart = [
                    sb.tile([128, NST, 2], f32, tag="mpart", bufs=4,
                            name=f"mp{grp}_{half}")
                    for half in range(2)
                ]

                def emit_sub(q4, cp):
                    for qt in range(q4 * 4, q4 * 4 + 4):
                        ps2 = []
                        for half in range(2):
                            off = 64 * half
                            ps = psp.tile([128, 1024], f32, tag="s1",
                                          bufs=2, name=f"ps_s1_{half}")
                            for c2 in range(2):
                                c = cp * 2 + c2
                                nc.tensor.matmul(
                                    ps[:, c2 * 512 : (c2 + 1) * 512],
                                    lhsT=ext2q[off : off + HD,
                                               qt * 128 : (qt + 1) * 128],
                                    rhs=ext2k[off : off + HD,
                                              c * 512 : (c + 1) * 512],
                                    start=True,
                                    stop=True,
                                )
                            ps2.append(ps)
                        for half in range(2):
                            nc.vector.reduce_max(
                                out=mpart[half][:, qt, cp : cp + 1],
                                in_=ps2[half],
                                axis=mybir.AxisListType.X,
                            )

                def finalize(hq):
                    """-m for q-tiles [hq*8, hq*8+8) of both heads — lets
                    pass 2's qh=hq chunks start after half the reduces."""
                    qsl = slice(hq * 8, hq * 8 + 8)
                    for half in range(2):
                        mcol = sb.tile([128, 8], f32, tag="mcol", bufs=4,
                                       name=f"mc{grp}_{half}_{hq}")
                        nc.vector.reduce_max(out=mcol,
                                             in_=mpart[half][:, qsl, :],
                                             axis=mybir.AxisListType.X)
                        pst = psp.tile([8, 128], f32, tag="s1", bufs=2,
                                       name=f"tp{grp}_{half}_{hq}")
                        nc.tensor.transpose(pst, mcol, ident)
                        msb = sb.tile([8, 128], f32r, tag="msb", bufs=4,
                                      name=f"ms{grp}_{half}_{hq}")
                        nc.vector.tensor_scalar_mul(msb, pst, -1.0)
                        mrow_d = dram.tile([1, 1024], f32r, tag="mrow",
                                           name=f"mrow{grp}_{half}_{hq}")
                        nc.sync.dma_start(
                            out=mrow_d[:].rearrange("o (a b) -> (o a) b",
                                                    a=8),
                            in_=msb,
                        )
                        nc.sync.dma_start(
                            out=qx_by_half[half][HD : HD + 1,
                                                 hq * 1024 : (hq + 1) * 1024],
                            in_=mrow_d[:],
                        )

                blocks = {
                    (q4, cp): (lambda q4=q4, cp=cp: emit_sub(q4, cp))
                    for q4 in range(4)
                    for cp in range(2)
                }
                blocks["finA"] = lambda: finalize(0)
                blocks["finB"] = lambda: finalize(1)
                return blocks

            def pass2_chunks(h, qx, kx, recb_on_act=False):
                """f32r S^T -> exp -> bf16 P^T -> PV (bf16) -> merged^T.
                Returns [2 qh-half closures]."""
                ch, offr = h // 2, 64 * (h % 2)

                state = {}

                def emit_sc(qh, kb):
                    # scores + exp for 4 k-tiles; their PV matmuls are a
                    # separate closure so another head's scores can slot in
                    # between (keeps ACT fed while PE does PV work)
                    if kb == 0:
                        state[qh] = [
                            psp.tile([128, 512], f32, tag="pv", bufs=2,
                                     name=f"pv{h}_{qh}_{c}")
                            for c in range(2)
                        ]
                    pts = []
                    for kt4 in range(4):
                        kt = kb * 4 + kt4
                        pt = sb.tile([128, 1024], bf16, tag="pt", bufs=10,
                                     name=f"pt{h}_{qh}_{kt}")
                        for c in range(2):
                            q0 = qh * 1024 + c * 512
                            ps = psp.tile([128, 512], f32, tag="s2",
                                          bufs=2, name="ps_s2")
                            nc.tensor.matmul(
                                ps,
                                lhsT=kx[:, kt * 128 : (kt + 1) * 128],
                                rhs=qx[:, q0 : q0 + 512],
                                start=True,
                                stop=True,
                            )
                            nc.scalar.activation(
                                out=pt[:, c * 512 : (c + 1) * 512],
                                in_=ps,
                                func=mybir.ActivationFunctionType.Exp,
                            )
                        pts.append((kt, pt))
                    state[(qh, kb)] = pts

                def emit_pv(qh, kb):
                    pv = state[qh]
                    for kt, pt in state.pop((qh, kb)):
                        for c in range(2):
                            nc.tensor.matmul(
                                pv[c][: HD + 1, :],
                                lhsT=v_sb[:, kt, h, :],
                                rhs=pt[:, c * 512 : (c + 1) * 512],
                                start=(kt == 0),
                                stop=(kt == NST - 1),
                            )

                def emit_fin(qh, c):
                    pv = state[qh]
                    q0 = qh * 1024 + c * 512
                    recb = sb.tile([HD, 512], f32r, tag="recb", bufs=2,
                                   name=f"rb{h}_{qh}_{c}")
                    with nc.allow_low_precision(
                        reason="1/l in f32r (19-bit) is plenty"
                    ):
                        nc.vector.reciprocal(
                            out=recb[0:1, :],
                            in_=pv[c][HD : HD + 1, :],
                        )
                    # replicate 1/l across 64 partitions (f32r matmul)
                    lps = psp.tile([HD, 512], f32, tag="s2", bufs=2,
                                   name=f"lp{h}_{qh}_{c}")
                    nc.tensor.matmul(lps, lhsT=ones1,
                                     rhs=recb[0:1, :],
                                     start=True, stop=True)
                    if recb_on_act:
                        nc.scalar.copy(out=recb, in_=lps)
                    else:
                        nc.vector.tensor_copy(out=recb, in_=lps)
                    nc.vector.tensor_mul(
                        out=mrg[ch][offr : offr + HD, q0 : q0 + 512],
                        in0=pv[c][0:HD, :],
                        in1=recb,
                    )

                return [
                    {
                        "sc": [lambda qh=qh, kb=kb: emit_sc(qh, kb)
                               for kb in range(NST // 4)],
                        "pv": [lambda qh=qh, kb=kb: emit_pv(qh, kb)
                               for kb in range(NST // 4)],
                        "fin": [lambda qh=qh, c=c: emit_fin(qh, c)
                                for c in range(2)],
                    }
                    for qh in range(2)
                ]

            def outproj_qt(qt, obuf_on_act=False):
                """one q-tile of the output projection (bf16 matmul)."""
                if True:
                    for n in range(2):
                        ps = psp.tile([128, 512], f32, tag="s1", bufs=2,
                                      name="ps_o")
                        for ch in range(2):
                            nc.tensor.matmul(
                                ps,
                                lhsT=mrg[ch][:, qt * 128 : (qt + 1) * 128],
                                rhs=wo_sb[:, ch, n * 512 : (n + 1) * 512],
                                start=(ch == 0),
                                stop=(ch == 1),
                            )
                        obuf = sb.tile([128, 512], f32, tag="obuf", bufs=2,
                                       name=f"ob{qt}_{n}")
                        if obuf_on_act:
                            nc.scalar.copy(out=obuf, in_=ps)
                        else:
                            nc.vector.tensor_copy(out=obuf, in_=ps)
                        nc.sync.dma_start(
                            out=out_d[qt * 128 : (qt + 1) * 128,
                                      n * 512 : (n + 1) * 512],
                            in_=obuf,
                        )

            # ---- pipeline ----
            # Emission order approximates the per-engine schedule with a
            # fine-grained round-robin: pass-1 sub-blocks (4 q-tiles, DVE
            # reduce work) alternate with projection/rope chunks and with
            # pass-2 kb batches (PE scores + ACT exp + PE PV), so no engine
            # waits long behind another's FIFO.
            qp0, kp0, proj0 = project_chunks(0)
            t0, rope0, ones_k0 = rope_pair(0)
            ext2q0, ext2k0, qx10, kx10 = t0
            proj0[0]()
            late_loads()
            rope0(0, qp0, kp0)
            proj0[1]()
            rope0(1, qp0, kp0)
            p1_0 = pass1_chunks(0, ext2q0, ext2k0, (ext2q0, qx10))
            qp1, kp1, proj1 = project_chunks(1)
            t1, rope1, ones_k1 = rope_pair(1)
            ext2q1, ext2k1, qx11, kx11 = t1
            # (q4,cp)=(*,0) needs only ext cols 0:1024 (sc 0-1)
            p1_0[(0, 0)]()
            p1_0[(1, 0)]()
            proj0[2]()
            rope0(2, qp0, kp0)
            p1_0[(2, 0)]()
            proj0[3]()
            rope0(3, qp0, kp0)
            p1_0[(3, 0)]()
            p1_0[(0, 1)]()
            proj1[0]()
            rope1(0, qp1, kp1)
            p1_0[(1, 1)]()
            p1_0["finA"]()
            proj1[1]()
            rope1(1, qp1, kp1)
            p1_0[(2, 1)]()
            proj1[2]()
            rope1(2, qp1, kp1)
            p1_0[(3, 1)]()
            p1_0["finB"]()
            ones_k0()  # ext2k0 row 64 -> ones (after pass 1 reads it)
            proj1[3]()
            rope1(3, qp1, kp1)

            p1_1 = pass1_chunks(1, ext2q1, ext2k1, (ext2q1, qx11))
            p2_0a = pass2_chunks(1, qx10, kx10, recb_on_act=True)
            p2_0b = pass2_chunks(0, ext2q0[0 : HD + 1, :],
                                 ext2k0[0 : HD + 1, :], recb_on_act=True)

            def unit(p, qh, fins=True):
                """software-pipelined (head, qh) group: scores run one kb
                batch ahead of their PV matmuls so PE never waits on exp"""
                sc, pv = p[qh]["sc"], p[qh]["pv"]
                seq = [sc[0], sc[1], pv[0], sc[2], pv[1], sc[3], pv[2], pv[3]]
                return seq + (p[qh]["fin"] if fins else [])

            # head 1's qh0 unit is ready as soon as finA lands — emit it
            # before grp1's pass 1 so its exps fill the ACT-idle lead-in
            for emit in unit(p2_0a, 0):
                emit()
            flatB = (unit(p2_0b, 0) + unit(p2_0a, 1) + unit(p2_0b, 1))
            subs1 = [(0, 0), (1, 0), (0, 1), (1, 1), "finA",
                     (2, 0), (3, 0), (2, 1), (3, 1), "finB"]
            fi = 0
            nsub = 0
            for key in subs1:
                p1_1[key]()
                if not isinstance(key, str):
                    n = 4 if nsub < 6 else 3
                    nsub += 1
                    for emit in flatB[fi : fi + n]:
                        emit()
                    fi += n
            for emit in flatB[fi:]:
                emit()
            ones_k1()  # ext2k1 row 64 -> ones (after pass 1 reads it)

            p2_1a = pass2_chunks(3, qx11, kx11)
            p2_1b = pass2_chunks(2, ext2q1[0 : HD + 1, :],
                                 ext2k1[0 : HD + 1, :])
            for emit in unit(p2_1a, 0) + unit(p2_1b, 0):
                emit()
            flatC = unit(p2_1a, 1) + unit(p2_1b, 1)
            for i in range(10):
                flatC[2 * i]()
                flatC[2 * i + 1]()
                if i < 8:
                    outproj_qt(i)
            for qt in range(8, NST):
                outproj_qt(qt, obuf_on_act=True)

    nc.compile()
    return nc


def _tables():
    j = np.arange(0, HD, 2, dtype=np.float32)
    inv_freq = (
        np.float32(1.0) / (np.float32(10000.0) ** (j / np.float32(HD)))
    ).astype(np.float32)
    freqs = np.arange(S, dtype=np.float32)[:, None] * inv_freq[None, :]  # [S, 32]
    cos = np.cos(freqs).astype(np.float32).T  # [32, S]
    sin = np.sin(freqs).astype(np.float32).T
    cos128 = np.concatenate([cos, cos, cos, cos], axis=0)  # [128, S]
    sin128 = np.concatenate([-sin, sin, -sin, sin], axis=0)
    return np.ascontiguousarray(cos128), np.ascontiguousarray(sin128)


def kernel(x, qw, kw, vw, ow):
    from concourse.bass_utils import run_bass_kernel_spmd

    if "nc" not in _STATE:
        _STATE["nc"] = _build()
    nc = _STATE["nc"]

    x = np.asarray(x, dtype=np.float32)
    qw = np.asarray(qw, dtype=np.float32)
    kw = np.asarray(kw, dtype=np.float32)
    vw = np.asarray(vw, dtype=np.float32)
    ow = np.asarray(ow, dtype=np.float32)

    cos64, sin64 = _tables()
    scale = np.float32(1.0 / 32.0)  # 1/sqrt(D), exact power of two

    in_maps = []
    for c in range(8):
        b, g = c // 4, c % 4
        sl = slice(g * GC, (g + 1) * GC)
        in_maps.append(
            {
                "xt": np.ascontiguousarray(x[b].T),
                "wq": np.ascontiguousarray(qw[:, sl]) * scale,
                "wk": np.ascontiguousarray(kw[:, sl]),
                "wv": np.ascontiguousarray(vw[:, sl]),
                "wo": np.ascontiguousarray(ow[sl, :]).astype(ml_dtypes.bfloat16),
                "onesrow": np.ones((1, S), dtype=np.float32),
                "cos64": cos64,
                "sin64": sin64,
            }
        )

    res = run_bass_kernel_spmd(nc, in_maps, core_ids=list(range(8)))
    _STATE["last_res"] = res
    outs = [r["out"] for r in res.results]
    full = np.empty((B, S, D), dtype=np.float32)
    for b in range(B):
        full[b] = sum(o.astype(np.float64) for o in outs[4 * b : 4 * b + 4]).astype(
            np.float32
        )
    return full
